# revision 7
# baseline (speedup 1.0000x reference)
"""Trainium2 Bass kernel for a 2-layer GAT + link predictor (nn_GAT).

Strategy (8 NeuronCores, SPMD single program):
  - Nodes are permuted and assigned to (core, rank) slots so every 128-rank
    block carries a near-equal number of incoming edges (load balance AND
    uniform program structure across cores).
  - Per layer:
      phase A (dense):  h = x @ W via PE; h rows written to DRAM.
      phase B (edges, sharded by dst): dma_gather h[src] rows; per-edge
        attention logits via on-chip dots + fp8 one-hot matmuls; softmax
        without max-subtraction (mathematically identical); weighted
        aggregation + denominator accumulate in PSUM via fp8 one-hot
        matmuls.  Per-core rank rows of s_dst are fetched with an indexed
        gather (per-core index inputs keep the program core-independent).
  - AllGather (DRAM collective) of layer outputs between layers.
  - Link predictor sharded over queries, 4-way grouped by (e0,e1) int16
    halves.
All index-derived data (permutations, one-hot tiles, gather indices) is
precomputed on the host from edge_index/edges; weights-derived scalars are
folded on the host (BN fold, W@a_dst).
"""
import heapq

import numpy as np
import ml_dtypes

from concourse import bass, bacc, tile
from concourse.bass_utils import run_bass_kernel_spmd

mybir = bass.mybir
AF = mybir.ActivationFunctionType
OP = mybir.AluOpType

F32 = mybir.dt.float32
BF16 = mybir.dt.bfloat16
F8 = mybir.dt.float8e4
I16 = mybir.dt.int16

N, C, E, Q = 50000, 128, 800000, 200000
NCORES = 8
NSH = N // NCORES            # 6250 nodes/core
RB = 128                     # ranks per block
NRB = (NSH + RB - 1) // RB   # 49 blocks/core
HALF = N // 2
RBS_PER_CHUNK = 3
BN_EPS = 1e-5

np_bf16 = ml_dtypes.bfloat16
np_f8 = ml_dtypes.float8_e4m3


# ----------------------------------------------------------------- host prep

def _build_perm(dst):
    """node -> (core, rank) assignment balancing edges per rank-block."""
    deg = np.bincount(dst, minlength=N)
    order = np.argsort(-deg, kind="stable")
    nblocks = NCORES * NRB
    caps = np.array([min(RB, NSH - rb * RB)
                     for k in range(NCORES) for rb in range(NRB)], np.int64)
    cnt = np.zeros(nblocks, np.int64)
    fill = [[] for _ in range(nblocks)]
    heap = [(0, b) for b in range(nblocks)]
    heapq.heapify(heap)
    for n in order:
        while True:
            load, b = heapq.heappop(heap)
            if cnt[b] < caps[b]:
                break
        fill[b].append(n)
        cnt[b] += 1
        if cnt[b] < caps[b]:
            heapq.heappush(heap, (load + int(deg[n]), b))
    perm = np.concatenate([np.array(fill[b], np.int64) for b in range(nblocks)])
    iperm = np.zeros(N, np.int64)
    iperm[perm] = np.arange(N)
    return perm, iperm


def _wrap_idx(idx):
    """[n] int -> [128, n//16] int16 (wrapped in 16 partitions, replicated)."""
    a = np.asarray(idx, np.int16).reshape(-1, 16).T
    return np.ascontiguousarray(np.tile(a, (8, 1)))


class _Tile:
    __slots__ = ("rb", "rbslot", "first", "last", "gslot")

    def __init__(self, rb, rbslot, first, last, gslot):
        self.rb, self.rbslot = rb, rbslot
        self.first, self.last = first, last
        self.gslot = gslot


class _Chunk:
    __slots__ = ("rbs", "tiles", "nA", "nB", "gcolA", "gcolB", "tilebase",
                 "rbcol")

    def __init__(self):
        self.rbs, self.tiles = [], []


def _prep(inputs):
    """All host-side index preprocessing. Returns a structure dict."""
    src = np.asarray(inputs["edge_index"][0], np.int64)
    dst = np.asarray(inputs["edge_index"][1], np.int64)
    perm, iperm = _build_perm(dst)
    src_p, dst_p = iperm[src], iperm[dst]

    dst_core = dst_p // NSH
    percore = []
    ntile_rbh = np.zeros((NRB, 2), np.int64)
    for k in range(NCORES):
        m = dst_core == k
        s, d = src_p[m], dst_p[m]
        rank = d - k * NSH
        rb = rank // RB
        half = (s >= HALF).astype(np.int64)
        percore.append((s, rank, rb, half))
        for r in range(NRB):
            c0 = int(((rb == r) & (half == 0)).sum())
            c1 = int(((rb == r) & (half == 1)).sum())
            ntile_rbh[r, 0] = max(ntile_rbh[r, 0], (c0 + 127) // 128)
            ntile_rbh[r, 1] = max(ntile_rbh[r, 1], (c1 + 127) // 128)
    ntile_rbh = np.maximum(ntile_rbh, 1)
    T = int(ntile_rbh.sum())

    # chunk structure (same for all cores)
    chunks = []
    tilebase = 0
    for c0 in range(0, NRB, RBS_PER_CHUNK):
        ch = _Chunk()
        ch.rbs = list(range(c0, min(c0 + RBS_PER_CHUNK, NRB)))
        ch.tilebase = tilebase
        gslot = 0
        for h in (0, 1):
            for rbslot, r in enumerate(ch.rbs):
                nt = int(ntile_rbh[r, h])
                for j in range(nt):
                    first = (h == 0 and j == 0)
                    last = (h == 1 and j == nt - 1)
                    ch.tiles.append(_Tile(r, rbslot, first, last, gslot))
                    gslot += 1
            if h == 0:
                ch.nA = gslot
        ch.nB = gslot - ch.nA
        tilebase += gslot
        chunks.append(ch)
    assert tilebase == T
    NTmax = max(ch.nA + ch.nB for ch in chunks)

    # gidx column ranges per (chunk, half): laid consecutively
    col = 0
    for ch in chunks:
        ch.gcolA = col
        col += ch.nA * 8
        ch.gcolB = col
        col += ch.nB * 8
    gcols = col
    # rb gather idx columns: 8 per rb, chunk slice = rbs
    for ch in chunks:
        ch.rbcol = ch.rbs[0] * 8

    # per-core edge data
    core_edge = []
    for k in range(NCORES):
        s, rank, rb, half = percore[k]
        gidx = np.zeros((128, gcols), np.int16)
        oh = np.zeros((128, T * 128), np_f8)
        ohT = np.zeros((128, T * 128), np_f8)
        for ch in chunks:
            for h in (0, 1):
                colbase = ch.gcolA if h == 0 else ch.gcolB
                slot0 = 0 if h == 0 else ch.nA
                nslots = ch.nA if h == 0 else ch.nB
                idxs = np.zeros(nslots * 128, np.int64)
                pos = 0
                for r in ch.rbs:
                    m2 = (rb == r) & (half == h)
                    ss = s[m2] - h * HALF
                    rk = rank[m2] - r * RB
                    nt = int(ntile_rbh[r, h])
                    idxs[pos:pos + len(ss)] = ss
                    # one-hot data, global tile index
                    gt0 = ch.tilebase + slot0 + pos // 128
                    e_in = np.arange(len(ss))
                    gpos = pos + e_in           # slot-local position
                    tt = ch.tilebase + slot0 + gpos // 128
                    pp = gpos % 128
                    oh[pp, tt * 128 + rk] = 1.0
                    ohT[rk, tt * 128 + pp] = 1.0
                    pos += nt * 128
                gidx[:, colbase:colbase + nslots * 8] = _wrap_idx(idxs)
        # rb row gathers: rank rows of this core, half-local with dummy 0
        base_rows = k * NSH + np.arange(NRB * RB)
        rbr = np.minimum(base_rows, (k + 1) * NSH - 1)  # clamp pad rows
        if k < 4:
            ra, rbb = rbr, np.zeros_like(rbr)
        else:
            ra, rbb = np.zeros_like(rbr), rbr - HALF
        core_edge.append(dict(
            gidx=gidx, oh=oh, ohT=ohT,
            rbidxA=_wrap_idx(ra), rbidxB=_wrap_idx(rbb),
        ))

    # ---- queries
    e0 = iperm[np.asarray(inputs["edges"][0], np.int64)]
    e1 = iperm[np.asarray(inputs["edges"][1], np.int64)]
    QSH = Q // NCORES
    qgrp_tiles = np.zeros(4, np.int64)
    qcore = []
    for k in range(NCORES):
        sl = slice(k * QSH, (k + 1) * QSH)
        a, b = e0[sl], e1[sl]
        g = (a >= HALF).astype(np.int64) * 2 + (b >= HALF).astype(np.int64)
        qcore.append((a, b, g))
        for gi in range(4):
            cnt = int((g == gi).sum())
            qgrp_tiles[gi] = max(qgrp_tiles[gi], (cnt + 127) // 128)
    QTT = int(qgrp_tiles.sum())
    # subchunks of <=32 tiles per group
    qchunks = []   # (group, tile0_in_out, ntiles)
    tpos = 0
    for gi in range(4):
        nt = int(qgrp_tiles[gi])
        j = 0
        while j < nt:
            step = min(32, nt - j)
            qchunks.append((gi, tpos + j, step))
            j += step
        tpos += nt

    core_q = []
    for k in range(NCORES):
        a, b, g = qcore[k]
        qi0 = np.zeros(QTT * 128, np.int64)
        qi1 = np.zeros(QTT * 128, np.int64)
        qmap = np.full(QTT * 128, -1, np.int64)
        tpos = 0
        for gi in range(4):
            m = g == gi
            cnt = int(m.sum())
            qi0[tpos:tpos + cnt] = a[m] - (gi >> 1) * HALF
            qi1[tpos:tpos + cnt] = b[m] - (gi & 1) * HALF
            qmap[tpos:tpos + cnt] = np.nonzero(m)[0] + k * QSH
            tpos += int(qgrp_tiles[gi]) * 128
        core_q.append(dict(qidx0=_wrap_idx(qi0), qidx1=_wrap_idx(qi1),
                           qmap=qmap))

    return dict(perm=perm, iperm=iperm, chunks=chunks, ntile_rbh=ntile_rbh,
                T=T, NTmax=NTmax, gcols=gcols, core_edge=core_edge,
                qgrp_tiles=qgrp_tiles, qchunks=qchunks, QTT=QTT,
                core_q=core_q)


def _rep(v):
    """[C] -> [128, C] replicated fp32."""
    return np.ascontiguousarray(np.broadcast_to(
        np.asarray(v, np.float32)[None, :], (128, C)))


# ------------------------------------------------------------ program build

def _build_program(S, fast1, bp2val, upto="ALL"):
    """Build the SPMD Bass program. S is the _prep structure.
    upto: stop after a stage ("A1","B1","AG1","A2","B2","AG2","ALL") for
    hardware bisection; dumps an intermediate into the dbg output."""
    nc = bacc.Bacc("TRN2", target_bir_lowering=False, debug=False,
                   num_devices=NCORES)
    T, NTmax, QTT = S["T"], S["NTmax"], S["QTT"]
    chunks, qchunks = S["chunks"], S["qchunks"]
    ntile_rbh = S["ntile_rbh"]

    def din(name, shape, dt):
        return nc.dram_tensor(name, shape, dt, kind="ExternalInput")

    # inputs
    embT = din("embT", [128, N], F32)
    W1 = din("W1", [128, C], F32)
    W2 = din("W2", [128, C], F32)
    a1rep = din("a1rep", [128, C], F32)
    a2rep = din("a2rep", [128, C], F32)
    adA1 = din("adA1", [128, C], F32)
    adB1 = din("adB1", [128, C], F32)
    adA2 = din("adA2", [128, C], F32)
    adB2 = din("adB2", [128, C], F32)
    Arep = din("Arep", [128, C], F32)
    C1rep = din("C1rep", [128, C], F32)
    b2rep = din("b2rep", [128, C], F32)
    Wp1 = din("Wp1", [128, C], F32)
    Wp2rep = din("Wp2rep", [128, C], F32)
    bp1rep = din("bp1rep", [128, C], F32)
    ident = din("ident", [128, 128], F32)
    ohA = din("ohA", [128, T * 128], F8)
    ohTA = din("ohTA", [128, T * 128], F8)
    gidx = din("gidx", [128, S["gcols"]], I16)
    rbidxA = din("rbidxA", [128, NRB * 8], I16)
    rbidxB = din("rbidxB", [128, NRB * 8], I16)
    qidx0 = din("qidx0", [128, QTT * 8], I16)
    qidx1 = din("qidx1", [128, QTT * 8], I16)

    out_q = nc.dram_tensor("out_q", [128, QTT], F32, kind="ExternalOutput")
    dbg = None
    if upto != "ALL":
        dbg = nc.dram_tensor("dbg", [128, NSH], F32, kind="ExternalOutput")

    # internal DRAM
    h_dram = nc.dram_tensor("h_dram", [N, C], F32)
    x1T_sh = nc.dram_tensor("x1T_sh", [128, NSH], F32)
    x1T_full = nc.dram_tensor("x1T_full", [NCORES * 128, NSH], F32,
                              addr_space="Shared")
    x2_sh = nc.dram_tensor("x2_sh", [NSH, C], F32)
    x2_full = nc.dram_tensor("x2_full", [N, C], F32, addr_space="Shared")

    with tile.TileContext(nc) as tc:
        with tc.tile_pool(name="const", bufs=1) as cpool:
            def load_const(t, w=C):
                sb = cpool.tile([128, w], F32, tag=t.name)
                nc.sync.dma_start(sb[:], t[:])
                return sb

            W1_sb = load_const(W1)
            W2_sb = load_const(W2)
            a1_sb = load_const(a1rep)
            a2_sb = load_const(a2rep)
            adA1_sb = load_const(adA1)
            adB1_sb = load_const(adB1)
            adA2_sb = load_const(adA2)
            adB2_sb = load_const(adB2)
            b2_sb = load_const(b2rep)
            Wp1_sb = load_const(Wp1)
            Wp2_sb = load_const(Wp2rep)
            bp1_sb = load_const(bp1rep)
            id_sb = load_const(ident, 128)
            if not fast1:
                A_sb = load_const(Arep)
                C1_sb = load_const(C1rep)
            gidx_sb = cpool.tile([128, S["gcols"]], I16)
            nc.sync.dma_start(gidx_sb[:], gidx[:])
            rbA_sb = cpool.tile([128, NRB * 8], I16)
            nc.sync.dma_start(rbA_sb[:], rbidxA[:])
            rbB_sb = cpool.tile([128, NRB * 8], I16)
            nc.sync.dma_start(rbB_sb[:], rbidxB[:])
            q0_sb = cpool.tile([128, QTT * 8], I16)
            nc.sync.dma_start(q0_sb[:], qidx0[:])
            q1_sb = cpool.tile([128, QTT * 8], I16)
            nc.sync.dma_start(q1_sb[:], qidx1[:])

            # ---------------- phase A: h = x @ W ----------------
            def phase_a(w_sb, layer):
                with tc.tile_pool(name="pa", bufs=3) as pa, \
                     tc.tile_pool(name="pap", bufs=2, space="PSUM") as pap:
                    # groups of up to 4 tiles of 128 nodes per input DMA
                    groups = []
                    if layer == 1:
                        n0 = 0
                        while n0 < N:
                            w = min(512, N - n0)
                            groups.append((n0, w))
                            n0 += w
                    else:
                        for ck in range(NCORES):
                            j = 0
                            while j < NSH:
                                w = min(512, NSH - j)
                                groups.append((ck * NSH + j, w))
                                j += w
                    for (n0, w) in groups:
                        xt = pa.tile([128, 512], F32, tag="pa_xt")
                        if layer == 1:
                            nc.sync.dma_start(xt[:, 0:w], embT[:, n0:n0 + w])
                        else:
                            ck, j = n0 // NSH, n0 % NSH
                            nc.sync.dma_start(
                                xt[:, 0:w],
                                x1T_full[ck * 128:(ck + 1) * 128, j:j + w])
                        nt = (w + 127) // 128
                        for t in range(nt):
                            tw = min(128, w - t * 128)
                            ps = pap.tile([128, C], F32, tag="pa_ps")
                            nc.tensor.matmul(ps[0:tw, :],
                                             xt[:, t * 128:t * 128 + tw],
                                             w_sb[:], start=True, stop=True)
                            hs = pa.tile([128, C], F32, tag="pa_hs")
                            nc.scalar.activation(hs[0:tw, :], ps[0:tw, :],
                                                 AF.Copy)
                            nc.sync.dma_start(
                                h_dram[n0 + t * 128:n0 + t * 128 + tw, :],
                                hs[0:tw, :])

            # ---------------- phase B: edge processing ----------------
            def phase_b(layer, a_sb, adA_sb, adB_sb):
                with tc.tile_pool(name="pb", bufs=2) as pb, \
                     tc.tile_pool(name="pbs", bufs=2) as pbs, \
                     tc.tile_pool(name="pbp", bufs=2, space="PSUM") as pbp:
                    trash = pbs.tile([128, 128], F32, tag="trash")
                    for ch in chunks:
                        nt = ch.nA + ch.nB
                        nrb = len(ch.rbs)
                        G = pb.tile([128, NTmax, C], F32, tag="G")
                        if ch.nA:
                            nc.gpsimd.dma_gather(
                                G[:, 0:ch.nA, :], h_dram[0:HALF, :],
                                gidx_sb[:, ch.gcolA:ch.gcolA + ch.nA * 8],
                                ch.nA * 128, ch.nA * 128, C, single_packet=False)
                        if ch.nB:
                            nc.gpsimd.dma_gather(
                                G[:, ch.nA:nt, :], h_dram[HALF:N, :],
                                gidx_sb[:, ch.gcolB:ch.gcolB + ch.nB * 8],
                                ch.nB * 128, ch.nB * 128, C, single_packet=False)
                        oh_sb = pb.tile([128, NTmax * 128], F8, tag="oh")
                        nc.sync.dma_start(
                            oh_sb[:, 0:nt * 128],
                            ohA[:, ch.tilebase * 128:(ch.tilebase + nt) * 128])
                        ohT_sb = pb.tile([128, NTmax * 128], F8, tag="ohT")
                        nc.sync.dma_start(
                            ohT_sb[:, 0:nt * 128],
                            ohTA[:, ch.tilebase * 128:(ch.tilebase + nt) * 128])
                        # per-core rank rows -> s_dst per rank block
                        sgA = pb.tile([128, RBS_PER_CHUNK, C], F32, tag="sgA")
                        sgB = pb.tile([128, RBS_PER_CHUNK, C], F32, tag="sgB")
                        nc.gpsimd.dma_gather(
                            sgA[:, 0:nrb, :], h_dram[0:HALF, :],
                            rbA_sb[:, ch.rbcol:ch.rbcol + nrb * 8],
                            nrb * 128, nrb * 128, C, single_packet=False)
                        nc.gpsimd.dma_gather(
                            sgB[:, 0:nrb, :], h_dram[HALF:N, :],
                            rbB_sb[:, ch.rbcol:ch.rbcol + nrb * 8],
                            nrb * 128, nrb * 128, C, single_packet=False)
                        sdb = pbs.tile([128, RBS_PER_CHUNK], BF16, tag="sdb")
                        for i in range(nrb):
                            sA = pbs.tile([128, 2], F32, tag="sA")
                            nc.vector.scalar_tensor_tensor(
                                trash[:], sgA[:, i, :], 1.0, adA_sb[:],
                                OP.mult, OP.mult, accum_out=sA[:, 0:1])
                            nc.vector.scalar_tensor_tensor(
                                trash[:], sgB[:, i, :], 1.0, adB_sb[:],
                                OP.mult, OP.mult, accum_out=sA[:, 1:2])
                            sc = pbs.tile([128, 1], F32, tag="sc")
                            nc.vector.tensor_add(sc[:], sA[:, 0:1], sA[:, 1:2])
                            nc.vector.tensor_copy(sdb[:, i:i + 1], sc[:])
                        # per-tile: s_src dot + s_dst matmul
                        ssrc = pbs.tile([128, NTmax], F32, tag="ssrc")
                        ps_sd = pbp.tile([128, NTmax], F32, tag="ps_sd")
                        for t, tl in enumerate(ch.tiles):
                            nc.vector.scalar_tensor_tensor(
                                trash[:], G[:, t, :], 1.0, a_sb[:],
                                OP.mult, OP.mult, accum_out=ssrc[:, t:t + 1])
                            nc.tensor.matmul(
                                ps_sd[:, t:t + 1],
                                ohT_sb[:, t * 128:(t + 1) * 128],
                                sdb[:, tl.rbslot:tl.rbslot + 1],
                                start=True, stop=True, skip_group_check=True)
                        z = pbs.tile([128, NTmax], F32, tag="z")
                        nc.vector.tensor_add(z[:, 0:nt], ssrc[:, 0:nt],
                                             ps_sd[:, 0:nt])
                        lr = pbs.tile([128, NTmax], F32, tag="lr")
                        # leaky relu on DVE: max(0.2*z, z)
                        nc.vector.scalar_tensor_tensor(
                            lr[:, 0:nt], z[:, 0:nt], 0.2, z[:, 0:nt],
                            OP.mult, OP.max)
                        w_sb = pbs.tile([128, NTmax], F32, tag="w")
                        nc.scalar.activation(w_sb[:, 0:nt], lr[:, 0:nt], AF.Exp)
                        rhs = pb.tile([128, NTmax, C + 1], BF16, tag="rhs")
                        for t in range(nt):
                            nc.scalar.activation(rhs[:, t, 0:C], G[:, t, :],
                                                 AF.Copy,
                                                 scale=w_sb[:, t:t + 1])
                        nc.vector.tensor_copy(rhs[:, 0:nt, C], w_sb[:, 0:nt])
                        ps_pack = pbp.tile([128, RBS_PER_CHUNK, C + 1], F32,
                                           tag="ps_pack")
                        for t, tl in enumerate(ch.tiles):
                            nc.tensor.matmul(
                                ps_pack[:, tl.rbslot, :],
                                oh_sb[:, t * 128:(t + 1) * 128],
                                rhs[:, t, :],
                                start=tl.first, stop=tl.last,
                                skip_group_check=True)
                        # epilogue per rank block
                        for i, r in enumerate(ch.rbs):
                            cap = min(RB, NSH - r * RB)
                            dn = pbs.tile([128, 1], F32, tag="dn")
                            nc.vector.tensor_scalar_add(
                                dn[:], ps_pack[:, i, C:C + 1], 1e-16)
                            rcp = pbs.tile([128, 1], F32, tag="rcp")
                            nc.vector.reciprocal(rcp[:], dn[:])
                            if layer == 1:
                                x1b = pbs.tile([128, C], F32, tag="x1b")
                                if fast1:
                                    nc.scalar.activation(
                                        x1b[:], ps_pack[:, i, 0:C], AF.Relu,
                                        scale=rcp[:])
                                else:
                                    y = pbs.tile([128, C], F32, tag="y")
                                    nc.vector.scalar_tensor_tensor(
                                        y[:], ps_pack[:, i, 0:C], rcp[:],
                                        A_sb[:], OP.mult, OP.mult)
                                    y2 = pbs.tile([128, C], F32, tag="y2")
                                    nc.vector.tensor_add(y2[:], y[:], C1_sb[:])
                                    nc.scalar.activation(x1b[:], y2[:],
                                                         AF.Relu)
                                ps_t = pbp.tile([128, 128], F32, tag="ps_t")
                                nc.tensor.transpose(ps_t[:], x1b[:], id_sb[:])
                                x1t = pbs.tile([128, 128], F32, tag="x1t")
                                nc.scalar.activation(x1t[:], ps_t[:], AF.Copy)
                                nc.sync.dma_start(
                                    x1T_sh[:, r * RB:r * RB + cap],
                                    x1t[:, 0:cap])
                            else:
                                x2b = pbs.tile([128, C], F32, tag="x2b")
                                nc.vector.scalar_tensor_tensor(
                                    x2b[:], ps_pack[:, i, 0:C], rcp[:],
                                    b2_sb[:], OP.mult, OP.add)
                                nc.sync.dma_start(
                                    x2_sh[r * RB:r * RB + cap, :],
                                    x2b[0:cap, :])

            # ---------------- run the stages ----------------
            def dump(src_ap, w):
                with tc.tile_pool(name="dump", bufs=1) as dp:
                    t = dp.tile([128, NSH], F32)
                    nc.gpsimd.memset(t[:], 0.0)
                    nc.sync.dma_start(t[:, 0:w], src_ap)
                    nc.sync.dma_start(dbg[:], t[:])

            done = False
            with nc.named_scope("A1"):
                phase_a(W1_sb, 1)
            if upto == "A1":
                dump(h_dram[0:128, :], 128)
                done = True
            if not done:
                with nc.named_scope("B1"):
                    phase_b(1, a1_sb, adA1_sb, adB1_sb)
                if upto == "B1":
                    dump(x1T_sh[:, 0:NSH], NSH)
                    done = True
            if not done:
                with nc.named_scope("AG1"):
                    nc.gpsimd.collective_compute(
                        "AllGather", OP.bypass,
                        replica_groups=[list(range(NCORES))],
                        ins=[x1T_sh.ap().opt()], outs=[x1T_full.ap().opt()])
                if upto == "AG1":
                    dump(x1T_full[128:256, 0:NSH], NSH)
                    done = True
            if not done:
                with nc.named_scope("A2"):
                    phase_a(W2_sb, 2)
                if upto == "A2":
                    dump(h_dram[0:128, :], 128)
                    done = True
            if not done:
                with nc.named_scope("B2"):
                    phase_b(2, a2_sb, adA2_sb, adB2_sb)
                if upto == "B2":
                    dump(x2_sh[0:128, :], 128)
                    done = True
            if not done:
                with nc.named_scope("AG2"):
                    nc.gpsimd.collective_compute(
                        "AllGather", OP.bypass,
                        replica_groups=[list(range(NCORES))],
                        ins=[x2_sh.ap().opt()], outs=[x2_full.ap().opt()])
                if upto == "AG2":
                    dump(x2_full[128:256, :], 128)
                    done = True

            # ---------------- phase C: link predictor ----------------
            with nc.named_scope("C"), \
                 tc.tile_pool(name="pc", bufs=2) as pc, \
                 tc.tile_pool(name="pcs", bufs=2) as pcs, \
                 tc.tile_pool(name="pcp", bufs=2, space="PSUM") as pcp:
                trash2 = pcs.tile([128, 128], F32, tag="trash2")
                for (gi, t0, nt) in (qchunks if not done else []):
                    b0 = (gi >> 1) * HALF
                    b1 = (gi & 1) * HALF
                    U = pc.tile([128, 32, C], F32, tag="U")
                    V = pc.tile([128, 32, C], F32, tag="V")
                    nc.gpsimd.dma_gather(
                        U[:, 0:nt, :], x2_full[b0:b0 + HALF, :],
                        q0_sb[:, t0 * 8:(t0 + nt) * 8],
                        nt * 128, nt * 128, C, single_packet=False)
                    nc.gpsimd.dma_gather(
                        V[:, 0:nt, :], x2_full[b1:b1 + HALF, :],
                        q1_sb[:, t0 * 8:(t0 + nt) * 8],
                        nt * 128, nt * 128, C, single_packet=False)
                    res = pcs.tile([128, 32], F32, tag="res")
                    for t in range(nt):
                        hq = pcs.tile([128, C], F32, tag="hq")
                        nc.vector.tensor_mul(hq[:], U[:, t, :], V[:, t, :])
                        ps_h = pcp.tile([128, 128], F32, tag="ps_h")
                        nc.tensor.transpose(ps_h[:], hq[:], id_sb[:])
                        ht = pcs.tile([128, 128], F32, tag="ht")
                        nc.scalar.activation(ht[:], ps_h[:], AF.Copy)
                        ps_z = pcp.tile([128, C], F32, tag="ps_z")
                        nc.tensor.matmul(ps_z[:], ht[:], Wp1_sb[:],
                                         start=True, stop=True)
                        zb = pcs.tile([128, C], F32, tag="zb")
                        nc.vector.scalar_tensor_tensor(
                            zb[:], ps_z[:], 1.0, bp1_sb[:], OP.mult, OP.add)
                        zr = pcs.tile([128, C], F32, tag="zr")
                        nc.scalar.activation(zr[:], zb[:], AF.Relu)
                        oc = pcs.tile([128, 1], F32, tag="oc")
                        nc.vector.scalar_tensor_tensor(
                            trash2[:], zr[:], 1.0, Wp2_sb[:],
                            OP.mult, OP.mult, accum_out=oc[:])
                        nc.scalar.activation(res[:, t:t + 1], oc[:],
                                             AF.Sigmoid, bias=float(bp2val))
                    nc.sync.dma_start(out_q[:, t0:t0 + nt],
                                      res[:, 0:nt])

    nc.compile()
    return nc


# ------------------------------------------------------------------- kernel

_CACHE = {}
LAST_RESULT = None


def build_all(inputs):
    """Host prep + program build + per-core input maps. Returns
    (nc, in_maps, S) for kernel() and for external bench harnesses."""
    inputs = {k: np.asarray(v) for k, v in inputs.items()}
    S = _prep(inputs)

    gamma = inputs["gamma"].astype(np.float32)
    rvar = inputs["rvar"].astype(np.float32)
    rmean = inputs["rmean"].astype(np.float32)
    beta = inputs["beta"].astype(np.float32)
    b1 = inputs["b1"].astype(np.float32)
    A = gamma / np.sqrt(rvar + BN_EPS)
    C1 = (b1 - rmean) * A + beta
    fast1 = bool(np.allclose(A, 1.0) and np.allclose(C1, 0.0))
    bp2val = float(np.asarray(inputs["bp2"]).reshape(-1)[0])

    nc = _build_program(S, fast1, bp2val)

    perm = S["perm"]
    emb = inputs["embedding"].astype(np.float32)
    embT_p = np.ascontiguousarray(emb[perm].T)

    W1 = inputs["W1"].astype(np.float32)
    W2 = inputs["W2"].astype(np.float32)
    maskA = np.zeros((128, 1), np.float32)
    maskB = np.zeros((128, 1), np.float32)

    common = dict(
        embT=embT_p,
        W1=W1, W2=W2,
        a1rep=_rep(inputs["a_src1"]),
        a2rep=_rep(inputs["a_src2"]),
        Arep=_rep(A), C1rep=_rep(C1),
        b2rep=_rep(inputs["b2"]),
        Wp1=inputs["Wp1"].astype(np.float32),
        Wp2rep=_rep(inputs["Wp2"][:, 0]),
        bp1rep=_rep(inputs["bp1"]),
        ident=np.eye(128, dtype=np.float32),
    )
    ad1 = _rep(inputs["a_dst1"])
    ad2 = _rep(inputs["a_dst2"])
    zer = np.zeros_like(ad1)

    in_maps = []
    for k in range(NCORES):
        ce, cq = S["core_edge"][k], S["core_q"][k]
        m = dict(common)
        if k < 4:
            m.update(adA1=ad1, adB1=zer, adA2=ad2, adB2=zer)
        else:
            m.update(adA1=zer, adB1=ad1, adA2=zer, adB2=ad2)
        m.update(ohA=ce["oh"], ohTA=ce["ohT"], gidx=ce["gidx"],
                 rbidxA=ce["rbidxA"], rbidxB=ce["rbidxB"],
                 qidx0=cq["qidx0"], qidx1=cq["qidx1"])
        in_maps.append(m)

    return nc, in_maps, S


def unpack_output(results, S):
    out = np.zeros(Q, np.float32)
    for k in range(NCORES):
        vals = np.asarray(results[k]["out_q"])      # [128, QTT]
        flat = vals.T.reshape(-1)                   # q = qt*128 + p
        qmap = S["core_q"][k]["qmap"]
        valid = qmap >= 0
        out[qmap[valid]] = flat[valid]
    return out


def kernel(**inputs):
    global LAST_RESULT
    nc, in_maps, S = build_all(inputs)
    res = run_bass_kernel_spmd(nc, in_maps, list(range(NCORES)))
    LAST_RESULT = res
    return unpack_output(res.results, S)

